# revision 3
# baseline (speedup 1.0000x reference)
"""H2GCNConv Trainium2 kernel: dual-hop SpMM via diagonal-packed gather + matmul.

Design:
  - Node-sharded: core c owns rows [c*6250, (c+1)*6250). No collectives.
  - Per hop, per core: rows sorted by degree (desc), grouped into 49 blocks of
    128. Edge slot (p, t) of a block holds the t-th edge of the block's p-th
    row, so the scatter is the IDENTITY: psum[p, :] += vals[p,t] * x[col[p,t]].
  - x[col] fetched with dma_gather (SWDGE): fp16 table, 256B descriptors,
    edges land on partitions. Full 50K-node range with int16 indices via a
    mid-table base: idx = col - 25000, src AP based at node 25000 (signed idx
    address math on the Q7 path).
  - Per tile: one scale op (vals, per-partition scalar) alternating DVE/ACT,
    one identity-lhsT matmul accumulating into the block's PSUM tile.
  - Host un-permutes the degree-sort and assembles [50000, 256] fp32.
"""

import sys
from contextlib import ExitStack

import numpy as np

sys.path.insert(0, "/opt/trn_rl_repo")

N_NODES = 50000
D_FEAT = 128
N_CORES = 8
RPC = N_NODES // N_CORES  # 6250 rows per core
NBLK = (RPC + 127) // 128  # 49 blocks (6272 padded rows)
BIAS = N_NODES // 2  # mid-table base for signed int16 gather indices
NUM_QUEUES = 2  # SWDGE queues for gather descriptor generation


def _block_groups(nblk):
    """Pair high-degree blocks with low-degree blocks to balance SBUF."""
    grps = []
    lo, hi = 0, nblk - 1
    while lo < hi:
        grps.append((lo, hi))
        lo += 1
        hi -= 1
    if lo == hi:
        grps.append((lo,))
    return grps


def _prep_hop(row, col, vals, n_nodes, rpc, n_cores, nblk):
    """Slot assignment for one hop. Returns per-core padded arrays + metadata."""
    row = np.asarray(row).astype(np.int64)
    col = np.asarray(col).astype(np.int64)
    vals = np.asarray(vals).astype(np.float32)
    core = row // rpc
    lrow = row % rpc
    key = core * rpc + lrow

    deg = np.bincount(key, minlength=n_cores * rpc)
    order = np.argsort(key, kind="stable")
    starts = np.zeros(n_cores * rpc + 1, dtype=np.int64)
    starts[1:] = np.cumsum(deg)
    rank = np.empty(len(row), dtype=np.int64)
    rank[order] = np.arange(len(row)) - starts[key[order]]

    degs = deg.reshape(n_cores, rpc)
    perm = np.argsort(-degs, axis=1, kind="stable")  # [c, s] -> lrow
    inv = np.empty_like(perm)
    np.put_along_axis(inv, perm, np.arange(rpc)[None, :].repeat(n_cores, 0), axis=1)
    s_pos = inv[core, lrow]  # sorted position of each edge's row

    # per-block tile counts, shared across cores (degree-desc => first row max)
    degs_sorted = np.take_along_axis(degs, perm, axis=1)  # [c, rpc] desc
    pad = np.zeros((n_cores, nblk * 128 - rpc), dtype=degs_sorted.dtype)
    degs_sorted = np.concatenate([degs_sorted, pad], axis=1)
    T_b = degs_sorted.reshape(n_cores, nblk, 128).max(axis=(0, 2))  # [nblk]

    b = s_pos // 128
    p = s_pos % 128
    t = rank  # t < deg(row) <= T_b[b]
    return dict(core=core, col=col, vals=vals, b=b, p=p, t=t, T_b=T_b, perm=perm)


def _prep(x, row1, col1, vals1, row2, col2, vals2, n_nodes=N_NODES, rpc=RPC,
          n_cores=N_CORES, nblk=NBLK, bias=BIAS):
    x = np.asarray(x).astype(np.float32)
    x16 = x.astype(np.float16)
    hops = [
        _prep_hop(row1, col1, vals1, n_nodes, rpc, n_cores, nblk),
        _prep_hop(row2, col2, vals2, n_nodes, rpc, n_cores, nblk),
    ]
    grps = _block_groups(nblk)

    # Global tile enumeration per hop, in program (group) order.
    tile_off = []  # per hop: [nblk] -> tile index offset within hop
    hop_tiles = []
    for h in range(2):
        off = np.zeros(nblk, dtype=np.int64)
        c = 0
        for grp in grps:
            for b in grp:
                off[b] = c
                c += int(hops[h]["T_b"][b])
        tile_off.append(off)
        hop_tiles.append(c)
    T_total = hop_tiles[0] + hop_tiles[1]
    hop_base = [0, hop_tiles[0]]

    # Batch layout: for each group, for each hop, the group's tiles chopped
    # into gather segments of <= SEG real tiles, each followed by one pad
    # tile (guarantees the last idx of every gather instruction is >= 0).
    # idx DRAM: [128, W] int16, slabs per (grp, hop) concatenated along free.
    SEG = 7
    batches = []  # (h, grp, nt_incl_pads, idx_col_off, segments, dst_pos)
    Wtot = 0
    for grp in grps:
        for h in range(2):
            nreal = int(sum(hops[h]["T_b"][b] for b in grp))
            dst_pos = {}  # (b, t) -> dst tile position
            segments = []  # (start_pos, end_pos_excl) incl pad tile
            pos = 0
            seg_start = 0
            seg_fill = 0
            for b in grp:
                for t in range(int(hops[h]["T_b"][b])):
                    dst_pos[(b, t)] = pos
                    pos += 1
                    seg_fill += 1
                    if seg_fill == SEG:
                        pos += 1  # pad tile
                        segments.append((seg_start, pos))
                        seg_start = pos
                        seg_fill = 0
            if seg_fill or nreal == 0:
                pos += 1
                segments.append((seg_start, pos))
            nt = pos
            batches.append((h, grp, nt, Wtot, segments, dst_pos))
            Wtot += nt * 8  # (nt*128)/16 int16 cols

    # Per-core flat idx array in batch order + vals [128, T_total].
    idx_flat = np.zeros((n_cores, Wtot * 16), dtype=np.int64)  # later int16
    vals_arr = np.zeros((n_cores, 128, max(T_total, 1)), dtype=np.float32)

    # position of tile g (global per-hop enum) inside the flat idx array:
    # batch offsets per (h, b): idx position = batch_off*16 + (local tile)*128 + p
    # flat idx position of each (h, b, t) tile, honouring pad-tile gaps
    tile_flat_pos = [dict(), dict()]  # per hop: (b, t) -> flat idx offset
    for (bh, grp, nt, coff, segments, dst_pos) in batches:
        for (b, t), p_ in dst_pos.items():
            tile_flat_pos[bh][(b, t)] = coff * 16 + p_ * 128

    for h in range(2):
        hp = hops[h]
        gidx = tile_off[h][hp["b"]] + hp["t"] + hop_base[h]  # vals col
        vals_arr[hp["core"], hp["p"], gidx] = hp["vals"]
        fp_lut = tile_flat_pos[h]
        base = np.empty(len(hp["b"]), dtype=np.int64)
        bb = hp["b"]; tt = hp["t"]
        # vectorize lookup: build array map [nblk, maxT]
        maxT = int(max((int(x) for x in hops[h]["T_b"]), default=0))
        lut = np.full((len(hops[h]["T_b"]), max(maxT, 1)), -1, dtype=np.int64)
        for (b_, t_), v_ in fp_lut.items():
            lut[b_, t_] = v_
        base = lut[bb, tt]
        assert (base >= 0).all()
        flat_pos = base + hp["p"]
        idx_flat[hp["core"], flat_pos] = hp["col"] - bias

    # wrap: j -> (j%16, j//16), replicated to 128 partitions
    idx_wrapped = np.zeros((n_cores, 128, Wtot), dtype=np.int16)
    w = idx_flat.reshape(n_cores, Wtot, 16).transpose(0, 2, 1).astype(np.int16)
    idx_wrapped[:] = np.tile(w, (1, 8, 1))

    cfg = dict(
        n_nodes=n_nodes, rpc=rpc, n_cores=n_cores, nblk=nblk, bias=bias,
        grps=grps, batches=batches, Wtot=Wtot, T_total=T_total,
        tile_off=tile_off, hop_base=hop_base,
        T_b=[hops[0]["T_b"], hops[1]["T_b"]],
    )
    in_maps = []
    ident = np.eye(128, dtype=np.float16)
    for c in range(n_cores):
        in_maps.append({
            "x16": x16,
            "idxs": idx_wrapped[c],
            "valsbuf": vals_arr[c],
            "ident": ident,
        })
    perms = [hops[0]["perm"], hops[1]["perm"]]
    return cfg, in_maps, perms


def _build(cfg, debug=False):
    import concourse.bacc as bacc
    import concourse.bass as bass
    import concourse.mybir as mybir
    import concourse.tile as tile

    f16 = mybir.dt.float16
    f32 = mybir.dt.float32
    i16 = mybir.dt.int16

    n_nodes = cfg["n_nodes"]
    nblk = cfg["nblk"]
    bias = cfg["bias"]
    grps = cfg["grps"]
    batches = cfg["batches"]
    T_b = cfg["T_b"]
    tile_off = cfg["tile_off"]
    hop_base = cfg["hop_base"]

    nc = bacc.Bacc("TRN2", target_bir_lowering=False, debug=debug,
                   num_devices=cfg["n_cores"], num_swdge_queues=NUM_QUEUES,
                   dynamic_dma_scratch_size=98304)

    x16 = nc.dram_tensor("x16", [n_nodes, D_FEAT], f16, kind="ExternalInput")
    idxs = nc.dram_tensor("idxs", [128, cfg["Wtot"]], i16, kind="ExternalInput")
    valsb = nc.dram_tensor("valsbuf", [128, max(cfg["T_total"], 1)], f32,
                           kind="ExternalInput")
    identd = nc.dram_tensor("ident", [128, 128], f16, kind="ExternalInput")
    outs = [
        nc.dram_tensor(f"out{h+1}", [nblk * 128, D_FEAT], f32,
                       kind="ExternalOutput")
        for h in range(2)
    ]

    # gather source based at node `bias` so signed int16 idx covers all nodes
    x_src = x16[bias:, :] if bias > 0 else x16[:, :]

    with tile.TileContext(nc) as tc, ExitStack() as ctx:
        const_pool = ctx.enter_context(tc.tile_pool(name="const", bufs=1))
        idx_pool = ctx.enter_context(tc.tile_pool(name="idx", bufs=3))
        g_pools = [
            ctx.enter_context(tc.tile_pool(name=f"gath{h}", bufs=2))
            for h in range(2)
        ]
        sc_pool = ctx.enter_context(tc.tile_pool(name="scaled", bufs=8))
        ps_pool = ctx.enter_context(tc.tile_pool(name="psum", bufs=4, space="PSUM"))
        st_pool = ctx.enter_context(tc.tile_pool(name="stage", bufs=4))

        ident_sb = const_pool.tile([128, 128], f16)
        nc.sync.dma_start(ident_sb[:, :], identd[:, :])
        vals_sb = const_pool.tile([128, max(cfg["T_total"], 1)], f32)
        nc.sync.dma_start(vals_sb[:, :], valsb[:, :])

        # batch lookup: (grp_index, h) -> (ntiles, col_off, segments, dst_pos)
        binfo = {}
        bi = 0
        for gi in range(len(grps)):
            for h in range(2):
                bh, grp, nt, coff, segments, dst_pos = batches[bi]
                assert bh == h and grp == grps[gi]
                binfo[(gi, h)] = (nt, coff, segments, dst_pos)
                bi += 1

        eng_flip = 0
        qn = 0
        for gi, grp in enumerate(grps):
            dsts = {}
            dposs = {}
            for h in range(2):
                nt, coff, segments, dst_pos = binfo[(gi, h)]
                w = nt * 8
                it = idx_pool.tile([128, w], i16, tag=f"idx{h}")
                nc.sync.dma_start(it[:, :], idxs[:, coff:coff + w])
                dst = g_pools[h].tile([128, nt, 128], f16, tag=f"gath{h}")
                for (a, bnd) in segments:
                    nidx = (bnd - a) * 128
                    nc.gpsimd.dma_gather(dst[:, a:bnd, :], x_src,
                                         it[:, a * 8:bnd * 8], nidx, nidx,
                                         128, queue_num=qn % NUM_QUEUES)
                    qn += 1
                dsts[h] = dst
                dposs[h] = dst_pos
            for h in range(2):
                for b in grp:
                    tb = int(T_b[h][b])
                    if tb == 0:
                        continue
                    psum = ps_pool.tile([128, 128], f32)
                    for t in range(tb):
                        g = hop_base[h] + tile_off[h][b] + t
                        sc = sc_pool.tile([128, 128], f16)
                        src_ap = dsts[h][:, dposs[h][(b, t)], :]
                        vap = vals_sb[:, g:g + 1]
                        if eng_flip % 2 == 0:
                            nc.vector.tensor_scalar_mul(sc[:, :], src_ap, vap)
                        else:
                            nc.scalar.mul(sc[:, :], src_ap, vap)
                        eng_flip += 1
                        nc.tensor.matmul(psum[:, :], ident_sb[:, :],
                                         sc[:, :], start=(t == 0),
                                         stop=(t == tb - 1))
                    stage = st_pool.tile([128, 128], f32)
                    nc.scalar.copy(stage[:, :], psum[:, :])
                    nc.sync.dma_start(outs[h][b * 128:(b + 1) * 128, :],
                                      stage[:, :])
    nc.finalize()
    return nc


def _run(inputs, trace=False, debug=False):
    from concourse.bass_utils import run_bass_kernel_spmd

    cfg, in_maps, perms = _prep(**inputs)
    nc = _build(cfg, debug=debug)
    res = run_bass_kernel_spmd(nc, in_maps, core_ids=list(range(cfg["n_cores"])),
                               trace=trace)
    rpc = cfg["rpc"]
    out = np.zeros((cfg["n_nodes"], 2 * D_FEAT), dtype=np.float32)
    for c in range(cfg["n_cores"]):
        for h in range(2):
            dev = res.results[c][f"out{h+1}"][:rpc, :]
            rows = c * rpc + perms[h][c]  # dev row s -> original local row
            out[rows, h * D_FEAT:(h + 1) * D_FEAT] = dev
    return out, res


def kernel(**inputs):
    out, _ = _run(inputs, trace=False)
    return out

